# revision 11
# baseline (speedup 1.0000x reference)
"""Trainium2 Bass kernel for nn_DisAttLayer (disentangled-attention bias MLP), v4.

Math (reference):
    e[b,m,n,h,:] = concat(pe[m-n+S], bi[b,m], bj[b,n], ci[b,m], cj[b,n])  (96)
    h1 = relu(e @ w1[:, :, h])     (96->32, per head)
    h2 = relu(h1 @ w2[:, :, h])    (32->16)
    score[b,h,m,n] = h2 @ w3[:, h] (16->1)

Key factorization: layer 1 is linear in the concat, so
    h1pre[b,m,n,h,k] = Ap[m-n+S,h,k] + Arow[b,m,h,k] + Acol[b,n,h,k].
With the free axis taken as n' = 255-n, the relative-position gather
Ap[m-n+S] becomes a sliding window over a 384-wide table.

v4 changes vs v2:
  - Ap/Acol/Arow/w2f tables are precomputed on the host (tiny), so the
    device program is just input DMA + the main loop; startup drops ~5us.
  - score output is DMA'd straight from PSUM (fp32) to DRAM - no
    PSUM->SBUF drain ops on ACT/DVE.
  - bias_relu DVE/ACT split re-tuned for the new balance (10/6 per it).

Stage-1 structure (per it = 8 m-values, per g = 4-head group):
  tmp = Ap_win + Acol_rep          (one DVE tensor_tensor, [128,2048], 2x mode)
  h1  = relu(tmp + Arow[m])        (16 biased tensor_scalar [128,256] ops,
                                    split ~10 DVE / ~6 ACT)

Sharding: 8 cores = batch b (4) x query-half m (2).
"""

import os
from contextlib import ExitStack

import numpy as np

import concourse.bacc as bacc
import concourse.bass as bass
import concourse.tile as tile
from concourse import mybir
from concourse.bass_utils import run_bass_kernel_spmd

S = 256
H = 8
B = 4
MH = 128          # m-values per core
APW = 392         # Ap window table width (cols m0 + m_local + j, < 383; padded)

F32 = mybir.dt.float32
F16 = mybir.dt.float16
BF16 = mybir.dt.bfloat16

N_WARMUP_MM = 12


def _declare_io(nc):
    def inp(name, shape, dt=F16):
        return nc.dram_tensor(name, list(shape), dt, kind="ExternalInput").ap()

    ins = {
        "Ap": inp("Ap", (2, 128, APW)),
        "Acol": inp("Acol", (2, 128, S)),          # n-reversed
        "Arow": inp("Arow", (2, 128, MH), F32),
        "w2f": inp("w2f", (128, 128)),
        "w3blk16": inp("w3blk16", (128, 2048)),
    }
    out = nc.dram_tensor("score4", [4, 128, 512], F16, kind="ExternalOutput").ap()
    return ins, out


# Per-(it, g, half) routing of the 4 bias_relu ops of each quarter tile:
# each entry gives how many of the 4 go to ACT (the rest stay on DVE).
# Balance target: ~6 of 16 per it on ACT.  ACT's own h2 work sits at the
# end of each half, so front-load its bias share a little.
def _mk_route():
    route = []
    for it in range(16):
        if it >= 14:
            # tail: keep ACT free to drain the last h2s quickly
            route.append([2, 1, 1, 0])
        elif it in (5, 10):
            route.append([2, 1, 1, 1])
        else:
            route.append([2, 1, 2, 1])
    return route


ROUTE = _mk_route()


def _emit(tc: tile.TileContext, X, out):
    nc = tc.nc
    AL = mybir.AluOpType
    AF = mybir.ActivationFunctionType

    with ExitStack() as ctx:
        const = ctx.enter_context(tc.tile_pool(name="const", bufs=1))

        # ---- input DMA (round-robin over queue engines) ----
        ld_n = [0]
        ld_eng = [nc.sync, nc.scalar, nc.gpsimd]

        def load(name, src, dt=F16):
            t = const.tile(list(src.shape), dt, name=f"sb_{name}")
            eng = ld_eng[ld_n[0] % len(ld_eng)]
            ld_n[0] += 1
            eng.dma_start(out=t, in_=src)
            return t

        Ap = [load(f"Ap{g}", X["Ap"][g]) for g in range(2)]
        Acol = [load(f"Acol{g}", X["Acol"][g]) for g in range(2)]
        Arow = [load(f"Arow{g}", X["Arow"][g], F32) for g in range(2)]
        w2f = load("w2f", X["w2f"])
        w3f = load("w3blk16", X["w3blk16"])

        # ---- PE warm-up: ramp the clock while input DMAs land ----
        with tc.tile_pool(name="psum_w", bufs=1, space="PSUM") as psum_w:
            warm_w = const.tile([128, 128], BF16, name="warm_w")
            warm_r = const.tile([128, 256], BF16, name="warm_r")
            nc.vector.memset(warm_w, 0.0)
            nc.vector.memset(warm_r, 0.0)
            ps_warm = psum_w.tile([128, 256], F32, name="ps_warm", tag="warm")
            for _ in range(N_WARMUP_MM):
                nc.tensor.matmul(out=ps_warm, lhsT=warm_w, rhs=warm_r,
                                 start=True, stop=True)

        # ---- main loop: 16 iterations x 8 m-values ----
        work = ctx.enter_context(tc.tile_pool(name="work", bufs=4))
        psum_m = ctx.enter_context(tc.tile_pool(name="psum_m", bufs=3, space="PSUM"))
        psum_o = ctx.enter_context(tc.tile_pool(name="psum_o", bufs=2, space="PSUM"))

        def bias_relu_dve(dst, src, bias_col):
            nc.vector.tensor_scalar(dst, src, bias_col, 0.0, AL.add, AL.max)

        def bias_relu_act(dst, src, bias_col):
            nc.scalar.activation(out=dst, in_=src, func=AF.Relu,
                                 bias=bias_col, scale=1.0)

        ps3 = None
        chunk = 0
        for it in range(MH // 8):
            m0 = 8 * it
            # h1 cols: 1024*(2g+half) + 256*mj + n'
            h1 = work.tile([128, 4096], F16, name=f"h1_{it}", tag="h1")
            tmps = []
            for g in range(2):
                # tmp = Ap_win + Acol_rep (sliding window over Ap; Acol read
                # through a repeating 2-dim AP, 256-wide runs)
                apg = Ap[g]
                apwin = bass.AP(
                    apg.tensor, apg.offset + m0,
                    [list(apg.ap[0]), [1, 8], [1, S]],
                )
                ac = Acol[g]
                acrep = bass.AP(ac.tensor, ac.offset,
                                [list(ac.ap[0]), [0, 8], [1, S]])
                tmp = work.tile([128, 2048], F16, name=f"tmp{g}_{it}", tag=f"tmp{g}")
                nc.vector.tensor_add(tmp, apwin, acrep)
                tmps.append(tmp)

            for g in range(2):
                for half in range(2):
                    q = 2 * g + half
                    o = 1024 * q
                    n_act = ROUTE[it][q]
                    for mj in range(4):
                        m = m0 + 4 * half + mj
                        src = tmps[g][:, 1024 * half + S * mj:
                                      1024 * half + S * (mj + 1)]
                        dst = h1[:, o + S * mj:o + S * (mj + 1)]
                        bias = Arow[g][:, m:m + 1]
                        # ACT (slower per op) takes the last m's so the
                        # c=0 matmul chunk (first consumer) is DVE-fed
                        if mj >= 4 - n_act:
                            bias_relu_act(dst, src, bias)
                        else:
                            bias_relu_dve(dst, src, bias)

            for half in range(2):
                ps2 = psum_m.tile([128, 1024], F32, name=f"ps2_{it}_{half}",
                                  tag="ps2")
                for g in range(2):
                    o = 1024 * (2 * g + half)
                    for c in range(2):
                        nc.tensor.matmul(
                            out=ps2[64 * g:64 * (g + 1), 512 * c:512 * (c + 1)],
                            lhsT=w2f[:, 64 * g:64 * (g + 1)],
                            rhs=h1[:, o + 512 * c:o + 512 * (c + 1)],
                            start=True, stop=True,
                        )

                h2 = work.tile([128, 1024], F16, name=f"h2_{it}_{half}", tag="h2")
                if it == 15 and half == 1:
                    # very tail: run the final relu on DVE so the two last
                    # half-chains drain on separate engines
                    nc.vector.tensor_scalar(out=h2[:, 0:512], in0=ps2[:, 0:512],
                                            scalar1=0.0, scalar2=None, op0=AL.max)
                    nc.vector.tensor_scalar(out=h2[:, 512:1024],
                                            in0=ps2[:, 512:1024],
                                            scalar1=0.0, scalar2=None, op0=AL.max)
                elif it >= 13:
                    nc.scalar.activation(out=h2[:, 0:512], in_=ps2[:, 0:512],
                                         func=AF.Relu)
                    nc.scalar.activation(out=h2[:, 512:1024], in_=ps2[:, 512:1024],
                                         func=AF.Relu)
                else:
                    nc.scalar.activation(out=h2, in_=ps2, func=AF.Relu)

                # stage 3: pack sixteen [8,512] score chunks into one PSUM
                # bank via dense zero-padded lhsT variants (rows 8q+h).
                for c in range(2):
                    q = chunk % 16
                    if q == 0:
                        ps3 = psum_o.tile([128, 512], F32,
                                          name=f"ps3_{chunk}", tag="ps3")
                    nc.tensor.matmul(out=ps3, lhsT=w3f[:, 128 * q:128 * (q + 1)],
                                     rhs=h2[:, 512 * c:512 * (c + 1)],
                                     start=(q == 0), stop=(q == 15))
                    if q == 15:
                        d = chunk // 16
                        sc = work.tile([128, 512], F16, name=f"sc_{d}", tag="sc")
                        if d == 3:
                            # last drain: keep it off ACT's tail queue
                            nc.vector.tensor_copy(sc, ps3)
                        else:
                            nc.scalar.activation(out=sc, in_=ps3, func=AF.Copy)
                        dma_eng = nc.sync if d % 2 == 0 else nc.gpsimd
                        dma_eng.dma_start(out=out[d], in_=sc)
                    chunk += 1


_PROGRAM = None


def _get_program():
    global _PROGRAM
    if _PROGRAM is None:
        nc = bacc.Bacc("TRN2", debug=False, num_devices=8)
        ins, out = _declare_io(nc)
        with tile.TileContext(nc) as tc:
            _emit(tc, ins, out)
        nc.compile()
        _PROGRAM = nc
    return _PROGRAM


def _build_in_maps(inputs):
    b_seq = np.asarray(inputs["b_seq"]).astype(np.int64)
    c_seq = np.asarray(inputs["c_seq"]).astype(np.int64)
    e_pos = np.asarray(inputs["e_pos"]).astype(np.float32)   # (512, 8, 32)
    e_bi = np.asarray(inputs["e_bi"]).astype(np.float32)     # (11, 8, 16)
    e_bj = np.asarray(inputs["e_bj"]).astype(np.float32)
    e_ci = np.asarray(inputs["e_ci"]).astype(np.float32)     # (102, 8, 16)
    e_cj = np.asarray(inputs["e_cj"]).astype(np.float32)
    w1 = np.asarray(inputs["w1_e"]).astype(np.float32)       # (96, 32, 8)
    w2 = np.asarray(inputs["w2_e"]).astype(np.float32)       # (32, 16, 8)
    w3 = np.asarray(inputs["w3_e"]).astype(np.float32)       # (16, 8)

    # ---- host precompute of the stage-1 tables ----
    # Ap_full[h,k,r] = sum_d w1[d,k,h] * e_pos[r,h,d] (emb fp16-rounded)
    e_pos16 = e_pos.astype(np.float16).astype(np.float32)
    ApH = np.einsum("dkh,rhd->hkr", w1[0:32], e_pos16)       # (8,32,512)
    ApHp = ApH.reshape(2, 128, 512)
    ApHp = np.concatenate(
        [ApHp, np.zeros((2, 128, APW + 8), np.float32)], axis=-1)

    def ttab(emb, w1rows):
        e16 = emb.astype(np.float16).astype(np.float32)
        return np.einsum("dkh,vhd->vhk", w1rows, e16)        # (V,8,32)

    Tbi = ttab(e_bi, w1[32:48])
    Tbj = ttab(e_bj, w1[48:64])
    Tci = ttab(e_ci, w1[64:80])
    Tcj = ttab(e_cj, w1[80:96])

    w2f = np.zeros((128, 128), np.float16)
    for g in range(2):
        for hh in range(4):
            w2f[32 * hh:32 * (hh + 1), 64 * g + 16 * hh:64 * g + 16 * (hh + 1)] = \
                w2[:, :, 4 * g + hh]
    w3blk16 = np.zeros((128, 2048), np.float16)
    for qv in range(16):
        for h in range(H):
            g, hh = h // 4, h % 4
            w3blk16[64 * g + 16 * hh:64 * g + 16 * hh + 16,
                    128 * qv + 8 * qv + h] = w3[:, h]

    shared = {"w2f": w2f, "w3blk16": w3blk16}

    in_maps = []
    for core in range(8):
        b, halfc = core // 2, core % 2
        m_off = halfc * MH
        im = dict(shared)
        # tile col t of Ap = Ap_full col (m_off + 1 + t); device reads
        # cols m + j for m in [0,128), j in [0,256)
        base = m_off + 1
        im["Ap"] = np.ascontiguousarray(
            ApHp[:, :, base:base + APW].astype(np.float16))
        bs_rev = b_seq[b, ::-1]
        cs_rev = c_seq[b, ::-1]
        Acol = Tbj[bs_rev] + Tcj[cs_rev]                     # (256,8,32)
        Acol = Acol.transpose(1, 2, 0).reshape(2, 128, S)
        im["Acol"] = np.ascontiguousarray(Acol.astype(np.float16))
        bs_row = b_seq[b, m_off:m_off + MH]
        cs_row = c_seq[b, m_off:m_off + MH]
        Arow = Tbi[bs_row] + Tci[cs_row]                     # (128,8,32)
        Arow = Arow.transpose(1, 2, 0).reshape(2, 128, MH)
        im["Arow"] = np.ascontiguousarray(Arow.astype(np.float32))
        in_maps.append(im)
    return in_maps


def _decode_part(part):
    """score4 [4, 128, 512] (n-reversed cols, dense 8q+h rows) ->
    [H, MH, S] with n forward."""
    sp = np.empty((H, MH, S), np.float32)
    p = part.astype(np.float32).reshape(4, 16, 8, 2, 256)  # d, q, h, mm, n'
    for d in range(4):
        for q in range(16):
            gc = 16 * d + q
            it, half, c = gc // 4, (gc % 4) // 2, gc % 2
            m = 8 * it + 4 * half + 2 * c
            sp[:, m:m + 2, :] = p[d, q, :, :, ::-1]
    return sp


def _assemble(core_outs):
    score = np.empty((B, H, S, S), np.float32)
    for core in range(8):
        b, half = core // 2, core % 2
        score[b, :, half * MH:(half + 1) * MH, :] = _decode_part(
            core_outs[core]["score4"])
    return score


def kernel(**inputs) -> np.ndarray:
    in_maps = _build_in_maps(inputs)
    nc = _get_program()

    if os.environ.get("BASSK_SIM"):
        from concourse.bass_interp import CoreSim
        score = np.zeros((B, H, S, S), np.float32)
        for core in [int(x) for x in os.environ["BASSK_SIM"].split(",")]:
            sim = CoreSim(nc, trace=False)
            for k, v in in_maps[core].items():
                sim.tensor(k)[:] = v
            sim.simulate(check_with_hw=False)
            b, half = core // 2, core % 2
            score[b, :, half * MH:(half + 1) * MH, :] = _decode_part(
                sim.tensor("score4").copy())
        return score

    res = run_bass_kernel_spmd(nc, in_maps, core_ids=list(range(8)))
    return _assemble(res.results)


# revision 12
# speedup vs baseline: 1.0224x; 1.0224x over previous
"""Trainium2 Bass kernel for nn_DisAttLayer (disentangled-attention bias MLP), v4.

Math (reference):
    e[b,m,n,h,:] = concat(pe[m-n+S], bi[b,m], bj[b,n], ci[b,m], cj[b,n])  (96)
    h1 = relu(e @ w1[:, :, h])     (96->32, per head)
    h2 = relu(h1 @ w2[:, :, h])    (32->16)
    score[b,h,m,n] = h2 @ w3[:, h] (16->1)

Key factorization: layer 1 is linear in the concat, so
    h1pre[b,m,n,h,k] = Ap[m-n+S,h,k] + Arow[b,m,h,k] + Acol[b,n,h,k].
With the free axis taken as n' = 255-n, the relative-position gather
Ap[m-n+S] becomes a sliding window over a 384-wide table.

v4 changes vs v2:
  - Ap/Acol/Arow/w2f tables are precomputed on the host (tiny), so the
    device program is just input DMA + the main loop; startup drops ~5us.
  - score output is DMA'd straight from PSUM (fp32) to DRAM - no
    PSUM->SBUF drain ops on ACT/DVE.
  - bias_relu DVE/ACT split re-tuned for the new balance (10/6 per it).

Stage-1 structure (per it = 8 m-values, per g = 4-head group):
  tmp = Ap_win + Acol_rep          (one DVE tensor_tensor, [128,2048], 2x mode)
  h1  = relu(tmp + Arow[m])        (16 biased tensor_scalar [128,256] ops,
                                    split ~10 DVE / ~6 ACT)

Sharding: 8 cores = batch b (4) x query-half m (2).
"""

import os
from contextlib import ExitStack

import numpy as np

import concourse.bacc as bacc
import concourse.bass as bass
import concourse.tile as tile
from concourse import mybir
from concourse.bass_utils import run_bass_kernel_spmd

S = 256
H = 8
B = 4
MH = 128          # m-values per core
APW = 392         # Ap window table width (cols m0 + m_local + j, < 383; padded)

F32 = mybir.dt.float32
F16 = mybir.dt.float16
BF16 = mybir.dt.bfloat16

N_WARMUP_MM = 12


def _declare_io(nc):
    def inp(name, shape, dt=F16):
        return nc.dram_tensor(name, list(shape), dt, kind="ExternalInput").ap()

    ins = {
        "Ap": inp("Ap", (2, 128, APW)),
        "Acol": inp("Acol", (2, 128, S)),          # n-reversed
        "Arow": inp("Arow", (2, 128, MH), F32),
        "w2f": inp("w2f", (128, 128)),
        "w3blk16": inp("w3blk16", (128, 2048)),
    }
    out = nc.dram_tensor("score4", [4, 128, 512], F16, kind="ExternalOutput").ap()
    return ins, out


# Per-(it, g, half) routing of the 4 bias_relu ops of each quarter tile:
# each entry gives how many of the 4 go to ACT (the rest stay on DVE).
# Balance target: ~6 of 16 per it on ACT.  ACT's own h2 work sits at the
# end of each half, so front-load its bias share a little.
def _mk_route():
    route = []
    for it in range(16):
        if it >= 14:
            # tail: keep ACT free to drain the last h2s quickly
            route.append([2, 1, 1, 0])
        elif it in (5, 10):
            route.append([2, 1, 1, 1])
        else:
            route.append([2, 1, 2, 1])
    return route


ROUTE = _mk_route()


def _emit(tc: tile.TileContext, X, out):
    nc = tc.nc
    AL = mybir.AluOpType
    AF = mybir.ActivationFunctionType

    with ExitStack() as ctx:
        const = ctx.enter_context(tc.tile_pool(name="const", bufs=1))

        # ---- input DMA (round-robin over queue engines) ----
        ld_n = [0]
        ld_eng = [nc.sync, nc.scalar, nc.gpsimd]

        def load(name, src, dt=F16):
            t = const.tile(list(src.shape), dt, name=f"sb_{name}")
            eng = ld_eng[ld_n[0] % len(ld_eng)]
            ld_n[0] += 1
            eng.dma_start(out=t, in_=src)
            return t

        Ap = [load(f"Ap{g}", X["Ap"][g]) for g in range(2)]
        Acol = [load(f"Acol{g}", X["Acol"][g]) for g in range(2)]
        Arow = [load(f"Arow{g}", X["Arow"][g], F32) for g in range(2)]
        w2f = load("w2f", X["w2f"])
        w3f = load("w3blk16", X["w3blk16"])

        # ---- PE warm-up: ramp the clock while input DMAs land ----
        with tc.tile_pool(name="psum_w", bufs=1, space="PSUM") as psum_w:
            warm_w = const.tile([128, 128], BF16, name="warm_w")
            warm_r = const.tile([128, 256], BF16, name="warm_r")
            nc.vector.memset(warm_w, 0.0)
            nc.vector.memset(warm_r, 0.0)
            ps_warm = psum_w.tile([128, 256], F32, name="ps_warm", tag="warm")
            for _ in range(N_WARMUP_MM):
                nc.tensor.matmul(out=ps_warm, lhsT=warm_w, rhs=warm_r,
                                 start=True, stop=True)

        # ---- main loop: 16 iterations x 8 m-values ----
        work = ctx.enter_context(tc.tile_pool(name="work", bufs=4))
        psum_m = ctx.enter_context(tc.tile_pool(name="psum_m", bufs=3, space="PSUM"))
        psum_o = ctx.enter_context(tc.tile_pool(name="psum_o", bufs=2, space="PSUM"))

        def bias_relu_dve(dst, src, bias_col):
            nc.vector.tensor_scalar(dst, src, bias_col, 0.0, AL.add, AL.max)

        def bias_relu_act(dst, src, bias_col):
            nc.scalar.activation(out=dst, in_=src, func=AF.Relu,
                                 bias=bias_col, scale=1.0)

        ps3 = None
        chunk = 0
        for it in range(MH // 8):
            m0 = 8 * it
            # h1 cols: 1024*(2g+half) + 256*mj + n'
            h1 = work.tile([128, 4096], F16, name=f"h1_{it}", tag="h1")
            tmps = []
            for g in range(2):
                # tmp = Ap_win + Acol_rep (sliding window over Ap; Acol read
                # through a repeating 2-dim AP, 256-wide runs)
                apg = Ap[g]
                apwin = bass.AP(
                    apg.tensor, apg.offset + m0,
                    [list(apg.ap[0]), [1, 8], [1, S]],
                )
                ac = Acol[g]
                acrep = bass.AP(ac.tensor, ac.offset,
                                [list(ac.ap[0]), [0, 8], [1, S]])
                tmp = work.tile([128, 2048], F16, name=f"tmp{g}_{it}", tag=f"tmp{g}")
                nc.vector.tensor_add(tmp, apwin, acrep)
                tmps.append(tmp)

            for g in range(2):
                for half in range(2):
                    q = 2 * g + half
                    o = 1024 * q
                    n_act = ROUTE[it][q]
                    for mj in range(4):
                        m = m0 + 4 * half + mj
                        src = tmps[g][:, 1024 * half + S * mj:
                                      1024 * half + S * (mj + 1)]
                        dst = h1[:, o + S * mj:o + S * (mj + 1)]
                        bias = Arow[g][:, m:m + 1]
                        if mj < n_act:
                            bias_relu_act(dst, src, bias)
                        else:
                            bias_relu_dve(dst, src, bias)

            for half in range(2):
                ps2 = psum_m.tile([128, 1024], F32, name=f"ps2_{it}_{half}",
                                  tag="ps2")
                for g in range(2):
                    o = 1024 * (2 * g + half)
                    for c in range(2):
                        nc.tensor.matmul(
                            out=ps2[64 * g:64 * (g + 1), 512 * c:512 * (c + 1)],
                            lhsT=w2f[:, 64 * g:64 * (g + 1)],
                            rhs=h1[:, o + 512 * c:o + 512 * (c + 1)],
                            start=True, stop=True,
                        )

                h2 = work.tile([128, 1024], F16, name=f"h2_{it}_{half}", tag="h2")
                if it == 15 and half == 1:
                    # very tail: run the final relu on DVE so the two last
                    # half-chains drain on separate engines
                    nc.vector.tensor_scalar(out=h2[:, 0:512], in0=ps2[:, 0:512],
                                            scalar1=0.0, scalar2=None, op0=AL.max)
                    nc.vector.tensor_scalar(out=h2[:, 512:1024],
                                            in0=ps2[:, 512:1024],
                                            scalar1=0.0, scalar2=None, op0=AL.max)
                elif it >= 13:
                    nc.scalar.activation(out=h2[:, 0:512], in_=ps2[:, 0:512],
                                         func=AF.Relu)
                    nc.scalar.activation(out=h2[:, 512:1024], in_=ps2[:, 512:1024],
                                         func=AF.Relu)
                else:
                    nc.scalar.activation(out=h2, in_=ps2, func=AF.Relu)

                # stage 3: pack sixteen [8,512] score chunks into one PSUM
                # bank via dense zero-padded lhsT variants (rows 8q+h).
                for c in range(2):
                    q = chunk % 16
                    if q == 0:
                        ps3 = psum_o.tile([128, 512], F32,
                                          name=f"ps3_{chunk}", tag="ps3")
                    nc.tensor.matmul(out=ps3, lhsT=w3f[:, 128 * q:128 * (q + 1)],
                                     rhs=h2[:, 512 * c:512 * (c + 1)],
                                     start=(q == 0), stop=(q == 15))
                    if q == 15:
                        d = chunk // 16
                        sc = work.tile([128, 512], F16, name=f"sc_{d}", tag="sc")
                        if d == 3:
                            # last drain: keep it off ACT's tail queue
                            nc.vector.tensor_copy(sc, ps3)
                        else:
                            nc.scalar.activation(out=sc, in_=ps3, func=AF.Copy)
                        dma_eng = nc.sync if d % 2 == 0 else nc.gpsimd
                        dma_eng.dma_start(out=out[d], in_=sc)
                    chunk += 1


_PROGRAM = None


def _get_program():
    global _PROGRAM
    if _PROGRAM is None:
        nc = bacc.Bacc("TRN2", debug=False, num_devices=8)
        ins, out = _declare_io(nc)
        with tile.TileContext(nc) as tc:
            _emit(tc, ins, out)
        nc.compile()
        _PROGRAM = nc
    return _PROGRAM


def _build_in_maps(inputs):
    b_seq = np.asarray(inputs["b_seq"]).astype(np.int64)
    c_seq = np.asarray(inputs["c_seq"]).astype(np.int64)
    e_pos = np.asarray(inputs["e_pos"]).astype(np.float32)   # (512, 8, 32)
    e_bi = np.asarray(inputs["e_bi"]).astype(np.float32)     # (11, 8, 16)
    e_bj = np.asarray(inputs["e_bj"]).astype(np.float32)
    e_ci = np.asarray(inputs["e_ci"]).astype(np.float32)     # (102, 8, 16)
    e_cj = np.asarray(inputs["e_cj"]).astype(np.float32)
    w1 = np.asarray(inputs["w1_e"]).astype(np.float32)       # (96, 32, 8)
    w2 = np.asarray(inputs["w2_e"]).astype(np.float32)       # (32, 16, 8)
    w3 = np.asarray(inputs["w3_e"]).astype(np.float32)       # (16, 8)

    # ---- host precompute of the stage-1 tables ----
    # Ap_full[h,k,r] = sum_d w1[d,k,h] * e_pos[r,h,d] (emb fp16-rounded)
    e_pos16 = e_pos.astype(np.float16).astype(np.float32)
    ApH = np.einsum("dkh,rhd->hkr", w1[0:32], e_pos16)       # (8,32,512)
    ApHp = ApH.reshape(2, 128, 512)
    ApHp = np.concatenate(
        [ApHp, np.zeros((2, 128, APW + 8), np.float32)], axis=-1)

    def ttab(emb, w1rows):
        e16 = emb.astype(np.float16).astype(np.float32)
        return np.einsum("dkh,vhd->vhk", w1rows, e16)        # (V,8,32)

    Tbi = ttab(e_bi, w1[32:48])
    Tbj = ttab(e_bj, w1[48:64])
    Tci = ttab(e_ci, w1[64:80])
    Tcj = ttab(e_cj, w1[80:96])

    w2f = np.zeros((128, 128), np.float16)
    for g in range(2):
        for hh in range(4):
            w2f[32 * hh:32 * (hh + 1), 64 * g + 16 * hh:64 * g + 16 * (hh + 1)] = \
                w2[:, :, 4 * g + hh]
    w3blk16 = np.zeros((128, 2048), np.float16)
    for qv in range(16):
        for h in range(H):
            g, hh = h // 4, h % 4
            w3blk16[64 * g + 16 * hh:64 * g + 16 * hh + 16,
                    128 * qv + 8 * qv + h] = w3[:, h]

    shared = {"w2f": w2f, "w3blk16": w3blk16}

    in_maps = []
    for core in range(8):
        b, halfc = core // 2, core % 2
        m_off = halfc * MH
        im = dict(shared)
        # tile col t of Ap = Ap_full col (m_off + 1 + t); device reads
        # cols m + j for m in [0,128), j in [0,256)
        base = m_off + 1
        im["Ap"] = np.ascontiguousarray(
            ApHp[:, :, base:base + APW].astype(np.float16))
        bs_rev = b_seq[b, ::-1]
        cs_rev = c_seq[b, ::-1]
        Acol = Tbj[bs_rev] + Tcj[cs_rev]                     # (256,8,32)
        Acol = Acol.transpose(1, 2, 0).reshape(2, 128, S)
        im["Acol"] = np.ascontiguousarray(Acol.astype(np.float16))
        bs_row = b_seq[b, m_off:m_off + MH]
        cs_row = c_seq[b, m_off:m_off + MH]
        Arow = Tbi[bs_row] + Tci[cs_row]                     # (128,8,32)
        Arow = Arow.transpose(1, 2, 0).reshape(2, 128, MH)
        im["Arow"] = np.ascontiguousarray(Arow.astype(np.float32))
        in_maps.append(im)
    return in_maps


def _decode_part(part):
    """score4 [4, 128, 512] (n-reversed cols, dense 8q+h rows) ->
    [H, MH, S] with n forward."""
    sp = np.empty((H, MH, S), np.float32)
    p = part.astype(np.float32).reshape(4, 16, 8, 2, 256)  # d, q, h, mm, n'
    for d in range(4):
        for q in range(16):
            gc = 16 * d + q
            it, half, c = gc // 4, (gc % 4) // 2, gc % 2
            m = 8 * it + 4 * half + 2 * c
            sp[:, m:m + 2, :] = p[d, q, :, :, ::-1]
    return sp


def _assemble(core_outs):
    score = np.empty((B, H, S, S), np.float32)
    for core in range(8):
        b, half = core // 2, core % 2
        score[b, :, half * MH:(half + 1) * MH, :] = _decode_part(
            core_outs[core]["score4"])
    return score


def kernel(**inputs) -> np.ndarray:
    in_maps = _build_in_maps(inputs)
    nc = _get_program()

    if os.environ.get("BASSK_SIM"):
        from concourse.bass_interp import CoreSim
        score = np.zeros((B, H, S, S), np.float32)
        for core in [int(x) for x in os.environ["BASSK_SIM"].split(",")]:
            sim = CoreSim(nc, trace=False)
            for k, v in in_maps[core].items():
                sim.tensor(k)[:] = v
            sim.simulate(check_with_hw=False)
            b, half = core // 2, core % 2
            score[b, :, half * MH:(half + 1) * MH, :] = _decode_part(
                sim.tensor("score4").copy())
        return score

    res = run_bass_kernel_spmd(nc, in_maps, core_ids=list(range(8)))
    return _assemble(res.results)
